# revision 11
# baseline (speedup 1.0000x reference)
"""Trainium2 Bass kernel: Luong-style attention with predictive alignment.

Math (see reference):
    h_t    = x[:, -1, :]                                   [B, H]
    t      = tanh(h_t @ W_p);  aligned = S*sigmoid(t @ v_p)
    scores[b,s] = sum_h x[b,s,h] * u[b,h],  u[b] = W_a @ h_t[b]
        (algebraic rewrite of (x @ W_a) . h_t -- avoids the B*S*H*H einsum)
    attn   = softmax(scores) * exp(-(pos-aligned)^2 / sigma2)
    ctx[b] = sum_s attn[b,s] * x[b,s,:]
    out    = tanh(concat(ctx, h_t) @ W_v)

Sharding: data-parallel over batch. 8 cores x 4 batches each; weights
replicated per core.

The kernel is DMA-roofline-shaped: 24.8MB/core (x fp16 16.8 + weights 8)
at ~360GB/s is ~69us, so every engine's per-batch work must fit under the
~11.7us/batch x-stream pace.  Three structural choices make that true:

1. Score dot products (the dominant elementwise work, 16 cols x
   [128,1024] per batch) are split across TWO engines: 10 columns on DVE
   and 6 on GpSimd (both run scalar_tensor_tensor; the op only has a 1x
   DVE uop so a second engine beats any single-engine scheme).
2. Context is accumulated PER CHUNK in PSUM with UNNORMALIZED bfloat16
   attention weights at = e^{s-m1} * G (bf16 carries fp32's exponent
   range, so the e^{+38} worst case cannot overflow; PE accepts mixed
   bf16 lhsT x fp16 rhs).  The 1/Z normalization folds into the single
   [1,H] PSUM->SBUF context copy at the end, and Z itself is reduced
   across partitions by a tiny PE matmul against a ones column instead
   of a gpsimd all-reduce.  This deletes the old 32-matmul post-zinv
   context burst from the tail.
3. u[b] broadcast: u = W_a h_t is computed once for all 4 batches as a
   [4,H] PE matmul, then broadcast to 128 partitions by a rank-1 PE
   matmul against a ones row - ~5us of PE instead of ~16us.

DMA ring order (sync queue): W_aT k0-3 | x b0c0 | W_aT k4-7 | x b0c1-2 |
W_p | x b0c3-7 | x b1 | x b2 | x b3 | W_v[:H] per-k | W_v[H:] per-k.
The ctx half of W_v streams first so its matmuls (gated on batch 3's
context, ready just after the last x tile) can chase the stream; the
always-ready h_t half lands last.  W_v halves reuse W_aT's and W_p's
SBUF slots.  Small inputs ride the otherwise idle gpsimd ring.

Exact softmax: e^{s-g2-m1}/sum(e^{s-m1}) == softmax(s)*gauss for any
bias m1 (we use max over cols 0:8, fixed after chunk 3).  sigmoid(z) is
1/(1+e^{-z}) so the scalar engine never swaps activation tables
(Sigmoid lives in a different table set than Exp/Square/Tanh).
"""

import math
from contextlib import ExitStack

import numpy as np

import concourse.bass as bass
import concourse.bass_isa as bass_isa
import concourse.mybir as mybir
import concourse.tile as tile
from concourse import bacc
from concourse.bass_utils import run_bass_kernel_spmd

B, S, H, SIZE = 32, 2048, 1024, 1024
NCORES = 8
BPC = B // NCORES          # batches per core
NCH = 8                    # x chunks per batch
SCH = S // NCH             # 256 sequence positions per chunk
A = 2                      # sub-slices (128 s-positions each) per chunk
COLS = NCH * A             # 16 score columns per batch
F32 = mybir.dt.float32
F16 = mybir.dt.float16
BF16 = mybir.dt.bfloat16
SIGMA_SQ = 2.0 * (S / 2.0 / 2.0) ** 2    # D = S//2; 2*(D/2)^2 = 524288
INV_SG = 1.0 / math.sqrt(SIGMA_SQ)

# score columns computed on gpsimd (rest on DVE); Pool rejects the
# scalar_tensor_tensor opcode on core V3, so this must stay empty
GP_COLS = frozenset()

_CACHE = {}
TRACE = False


def _build():
    AF = mybir.ActivationFunctionType
    OP = mybir.AluOpType
    nc = bacc.Bacc()

    x_s = nc.dram_tensor("x_s", [BPC, S, H], F16, kind="ExternalInput")
    w_p = nc.dram_tensor("w_p", [H, H], F16, kind="ExternalInput")
    w_at = nc.dram_tensor("w_at", [H, H], F16, kind="ExternalInput")
    w_v = nc.dram_tensor("w_v", [2 * H, SIZE], F16, kind="ExternalInput")
    htk = nc.dram_tensor("htk", [128, 8 * BPC], F16, kind="ExternalInput")
    vrep = nc.dram_tensor("vrep", [BPC, H], F32, kind="ExternalInput")
    posd = nc.dram_tensor("pos", [128, COLS + 1], F32, kind="ExternalInput")
    idd = nc.dram_tensor("ident", [128, 128], F32, kind="ExternalInput")
    seld = nc.dram_tensor("sel16", [BPC, BPC * 128], F16, kind="ExternalInput")
    outd = nc.dram_tensor("out", [BPC, SIZE], F32, kind="ExternalOutput")

    with tile.TileContext(nc) as tc, ExitStack() as ctx:
        const = ctx.enter_context(tc.tile_pool(name="const", bufs=1))
        wts = ctx.enter_context(tc.tile_pool(name="wts", bufs=1))
        xs = ctx.enter_context(tc.tile_pool(name="xs", bufs=14))
        ubcp = ctx.enter_context(tc.tile_pool(name="ubcp", bufs=4))
        ctxp = ctx.enter_context(tc.tile_pool(name="ctxp", bufs=2))
        prodp = ctx.enter_context(tc.tile_pool(name="prodp", bufs=2))
        prodg = ctx.enter_context(tc.tile_pool(name="prodg", bufs=2))
        small = ctx.enter_context(tc.tile_pool(name="small", bufs=2))
        psUbc = ctx.enter_context(
            tc.tile_pool(name="psUbc", bufs=1, space=bass.MemorySpace.PSUM)
        )
        psCtx = ctx.enter_context(
            tc.tile_pool(name="psCtx", bufs=1, space=bass.MemorySpace.PSUM)
        )
        psT = ctx.enter_context(
            tc.tile_pool(name="psT", bufs=1, space=bass.MemorySpace.PSUM)
        )
        psO = ctx.enter_context(
            tc.tile_pool(name="psO", bufs=1, space=bass.MemorySpace.PSUM)
        )
        dpool = ctx.enter_context(
            tc.tile_pool(name="dram", bufs=1, space=bass.MemorySpace.DRAM)
        )

        # ---- small inputs ride the gpsimd ring; bulk traffic owns sync ----
        combT = const.tile([128, 8 * BPC * 2], F16)  # combined^T: [p, 4k+b]
        v_sb = const.tile([BPC, H], F32)
        pos_sb = const.tile([128, COLS + 1], F32)    # last col = 1.0 (Z-sum)
        id_sb = const.tile([128, 128], F32)
        sel16 = const.tile([BPC, BPC * 128], F16)
        u16 = const.tile([BPC, H], F16)
        tta = const.tile([BPC, H], F32)
        alb = const.tile([BPC, 1], F32)
        out_sb = const.tile([BPC, SIZE], F32)

        nc.gpsimd.dma_start(out=combT[:, 32:64], in_=htk[:, :])
        nc.gpsimd.dma_start(out=v_sb, in_=vrep[:, :])
        nc.gpsimd.dma_start(out=pos_sb, in_=posd[:, :])
        nc.gpsimd.dma_start(out=id_sb, in_=idd[:, :])
        nc.gpsimd.dma_start(out=sel16, in_=seld[:, :])
        ones32 = pos_sb[:, COLS : COLS + 1]

        # ---- bulk ring: W_aT first half, then first x tile, etc ----
        all_x = [[None] * NCH for _ in range(BPC)]

        def emit_x_dmas(b, cs):
            for c in cs:
                xt = xs.tile([128, A, H], F16, tag="xt", name=f"xt_{b}_{c}")
                nc.sync.dma_start(
                    out=xt,
                    in_=x_s[b, c * SCH : (c + 1) * SCH, :]
                    .rearrange("(p a) h -> p a h", p=128),
                )
                all_x[b][c] = xt

        wa_sb = wts.tile([128, 8, H], F16, tag="w1")
        nc.sync.dma_start(
            out=wa_sb[:, 0:4, :],
            in_=w_at[0 : H // 2, :].rearrange("(k p) j -> p k j", p=128),
        )
        emit_x_dmas(0, [0])
        nc.sync.dma_start(
            out=wa_sb[:, 4:8, :],
            in_=w_at[H // 2 :, :].rearrange("(k p) j -> p k j", p=128),
        )

        # u = W_a @ h_t for all 4 batches at once: [4, H] PSUM
        u_ps = psO.tile([BPC, H], F32, tag="po", name="u_ps")
        for k in range(8):
            lhs = combT[:, 32 + 4 * k : 32 + 4 * k + 4]
            for h2 in range(2):
                nc.tensor.matmul(
                    u_ps[:, 512 * h2 : 512 * (h2 + 1)],
                    lhs,
                    wa_sb[:, k, 512 * h2 : 512 * (h2 + 1)],
                    start=(k == 0),
                    stop=(k == 7),
                )
        nc.scalar.copy(u16[:, 0 : H // 2], u_ps[:, 0 : H // 2])
        nc.vector.tensor_copy(out=u16[:, H // 2 : H], in_=u_ps[:, H // 2 : H])

        emit_x_dmas(0, [1, 2])
        wp_sb = wts.tile([128, 8, H], F16, tag="w0")
        nc.sync.dma_start(
            out=wp_sb, in_=w_p[:, :].rearrange("(k p) j -> p k j", p=128)
        )
        emit_x_dmas(0, range(3, NCH))

        # broadcast u[b] to all 128 partitions: selector matmul (row b of
        # sel16[:, 128b:128(b+1)] is all-ones) against u16 at base partition 0
        ubc_tiles = [None] * BPC
        for b in range(BPC):
            ub_ps = psUbc.tile([128, H], F32, tag="ub", name=f"ubps_{b}")
            for h2 in range(2):
                nc.tensor.matmul(
                    ub_ps[:, 512 * h2 : 512 * (h2 + 1)],
                    sel16[:, 128 * b : 128 * (b + 1)],
                    u16[:, 512 * h2 : 512 * (h2 + 1)],
                    start=True,
                    stop=True,
                )
            ubc = ubcp.tile([128, H], F16, tag="ubc", name=f"ubc_{b}")
            nc.scalar.copy(ubc[:, 0 : H // 2], ub_ps[:, 0 : H // 2])
            nc.vector.tensor_copy(out=ubc[:, H // 2 : H], in_=ub_ps[:, H // 2 : H])
            ubc_tiles[b] = ubc

        # ---- alignment: t = tanh(h_t@W_p); aligned = S*sigmoid(t@v_p) ----
        ab_d = dpool.tile([BPC, 1], F32)
        ab_tiles = [
            const.tile([128, 1], F32, name=f"abb_{bb}") for bb in range(BPC)
        ]

        def emit_aligned_section():
            ps_t = psO.tile([BPC, H], F32, tag="po", name="ps_t")
            for k in range(8):
                lhs = combT[:, 32 + 4 * k : 32 + 4 * k + 4]
                for h2 in range(2):
                    nc.tensor.matmul(
                        ps_t[:, 512 * h2 : 512 * (h2 + 1)],
                        lhs,
                        wp_sb[:, k, 512 * h2 : 512 * (h2 + 1)],
                        start=(k == 0),
                        stop=(k == 7),
                    )
            nc.scalar.activation(out=tta, in_=ps_t, func=AF.Tanh)

            prod0 = prodp.tile([BPC, H], F32, tag="pal")
            al_r = small.tile([BPC, 1], F32, tag="alr")
            nc.vector.scalar_tensor_tensor(
                out=prod0,
                in0=tta,
                scalar=1.0,
                in1=v_sb,
                op0=OP.mult,
                op1=OP.mult,
                accum_out=al_r,
            )
            # sigmoid via resident Exp table (avoids ACT_TABLE_LOAD swaps)
            e_neg = small.tile([BPC, 1], F32, tag="eneg")
            nc.scalar.activation(out=e_neg, in_=al_r, func=AF.Exp, bias=0.0, scale=-1.0)
            e_p1 = small.tile([BPC, 1], F32, tag="ep1")
            nc.vector.tensor_scalar_add(e_p1, e_neg, 1.0)
            sigv = small.tile([BPC, 1], F32, tag="sigv")
            nc.vector.reciprocal(sigv, e_p1)
            nc.scalar.mul(alb, sigv, -float(S) * INV_SG)  # alb = -aligned/sg
            # roundtrip + per-batch broadcasts on the idle gpsimd ring
            nc.gpsimd.dma_start(out=ab_d[:, :], in_=alb)
            for bb in range(BPC):
                nc.gpsimd.dma_start(
                    out=ab_tiles[bb], in_=ab_d[bb : bb + 1, :].to_broadcast((128, 1))
                )

        # ---- per-batch: scores + per-chunk unnormalized bf16 context ----
        NCH0 = 4          # chunks covered by the m1 bias phase
        C0 = NCH0 * A     # cols 0..7

        def batch_section(b, after_scores=None):
            ubc = ubc_tiles[b]
            sc_b = small.tile([128, COLS], F32, tag="scb", name=f"scb_{b}")
            ps_c = psCtx.tile([1, H], F32, tag="pc", name=f"pc_{b}")

            def emit_stt(col):
                c, a = col // A, col % A
                if col in GP_COLS:
                    eng, pool = nc.gpsimd, prodg
                else:
                    eng, pool = nc.vector, prodp
                prod = pool.tile([128, H], F16, tag="p0", name=f"pr_{b}_{col}")
                eng.scalar_tensor_tensor(
                    out=prod,
                    in0=all_x[b][c][:, a, :],
                    scalar=1.0,
                    in1=ubc,
                    op0=OP.mult,
                    op1=OP.mult,
                    accum_out=sc_b[:, col : col + 1],
                )

            def emit_ctx_mms(c, at_ap, first, last):
                # 4 matmuls per chunk: 2 sub-slice cols x 2 H-halves
                for a in range(A):
                    for h2 in range(2):
                        nc.tensor.matmul(
                            ps_c[:, 512 * h2 : 512 * (h2 + 1)],
                            at_ap[:, a : a + 1],
                            all_x[b][c][:, a, 512 * h2 : 512 * (h2 + 1)],
                            start=(first and a == 0),
                            stop=(last and a == 1 and h2 == 1),
                        )

            for col in range(C0):
                emit_stt(col)

            # bias m1 = max over cols 0..7, broadcast to all partitions
            mx_p = small.tile([128, 1], F32, tag="mxp", name=f"mxp_{b}")
            nc.vector.reduce_max(
                out=mx_p, in_=sc_b[:, 0:C0], axis=mybir.AxisListType.X
            )
            mcast = small.tile([128, 1], F32, tag="mcast", name=f"mcast_{b}")
            nc.gpsimd.partition_all_reduce(
                mcast, mx_p, channels=128, reduce_op=bass_isa.ReduceOp.max
            )
            negm = small.tile([128, 1], F32, tag="negm", name=f"negm_{b}")
            nc.scalar.mul(negm, mcast, -1.0)

            # lookahead: chunk 4's score STTs before any scalar-dependent op
            for col in range(C0, C0 + A):
                emit_stt(col)
            if after_scores is not None:
                after_scores()

            g2 = small.tile([128, COLS], F32, tag="g2", name=f"g2_{b}")
            nc.scalar.activation(
                out=g2, in_=pos_sb[:, 0:COLS], func=AF.Square,
                bias=ab_tiles[b], scale=INV_SG,
            )
            gss = small.tile([128, COLS], F32, tag="gss", name=f"gss_{b}")
            nc.scalar.activation(out=gss, in_=g2, func=AF.Exp, bias=0.0, scale=-1.0)

            ew_all = small.tile([128, COLS], F32, tag="ew", name=f"ew_{b}")
            zp0 = small.tile([128, 1], F32, tag="zp0", name=f"zp0_{b}")
            nc.scalar.activation(
                out=ew_all[:, 0:C0],
                in_=sc_b[:, 0:C0],
                func=AF.Exp,
                bias=negm,
                scale=1.0,
                accum_out=zp0,
            )
            # unnormalized bf16 attention weights for chunks 0-3; their
            # context matmuls fire now (e^{s-m1} <= 1 here by construction,
            # and bf16 has fp32's exponent range for the later chunks)
            at8 = small.tile([128, C0], BF16, tag="at8", name=f"at8_{b}")
            nc.vector.tensor_mul(at8, ew_all[:, 0:C0], gss[:, 0:C0])
            for c in range(NCH0):
                emit_ctx_mms(c, at8[:, A * c : A * (c + 1)], first=(c == 0), last=False)

            # chunks 4..7: per-chunk exp -> at -> context, with the NEXT
            # chunk's score STTs emitted first so the engines never stall
            zrun = zp0
            for ch in range(NCH0, NCH):
                col0 = ch * A
                if ch + 1 < NCH:
                    for col in range((ch + 1) * A, (ch + 2) * A):
                        emit_stt(col)
                zpc = small.tile([128, 1], F32, tag=f"zp{ch}", name=f"zp{ch}_{b}")
                nc.scalar.activation(
                    out=ew_all[:, col0 : col0 + A],
                    in_=sc_b[:, col0 : col0 + A],
                    func=AF.Exp,
                    bias=negm,
                    scale=1.0,
                    accum_out=zpc,
                )
                at2 = small.tile([128, A], BF16, tag=f"at{ch}", name=f"at{ch}_{b}")
                nc.vector.tensor_mul(
                    at2, ew_all[:, col0 : col0 + A], gss[:, col0 : col0 + A]
                )
                emit_ctx_mms(ch, at2, first=False, last=(ch == NCH - 1))
                zn = small.tile([128, 1], F32, tag=f"zr{ch}", name=f"zr{ch}_{b}")
                nc.scalar.add(zn, zpc, zrun)
                zrun = zn

            # Z = sum_p zrun[p] via a tiny PE matmul against the ones col
            psz = psT.tile([1, 1], F32, tag="pz", name=f"pz_{b}")
            nc.tensor.matmul(psz, zrun, ones32, start=True, stop=True)
            zinv = small.tile([1, 1], F32, tag="zinv", name=f"zinv_{b}")
            nc.vector.reciprocal(zinv, psz)

            # ctx out of PSUM with 1/Z folded in (split scalar/vector),
            # transpose 128-blocks, scatter into combT's per-batch columns
            ctx_t = ctxp.tile([1, H], F32, tag="ctx", name=f"ctx_{b}")
            nc.scalar.mul(ctx_t[0:1, 0 : H // 2], ps_c[0:1, 0 : H // 2], zinv)
            nc.vector.tensor_scalar(
                out=ctx_t[0:1, H // 2 : H],
                in0=ps_c[0:1, H // 2 : H],
                scalar1=zinv,
                scalar2=None,
                op0=OP.mult,
            )
            ps_ct = psT.tile([128, 8], F32, tag="pt", name=f"pct_{b}")
            for k in range(8):
                nc.tensor.transpose(
                    ps_ct[:, k : k + 1],
                    ctx_t[0:1, 128 * k : 128 * (k + 1)],
                    id_sb[0:1, 0:1],
                )
            cT = combT[:, b : b + 1]
            comb_cols = bass.AP(
                tensor=cT.tensor, offset=cT.offset, ap=[cT.ap[0], [4, 8]]
            )
            nc.scalar.copy(comb_cols, ps_ct)

        batch_section(0, after_scores=emit_aligned_section)
        emit_x_dmas(1, range(NCH))
        batch_section(1)
        emit_x_dmas(2, range(NCH))
        batch_section(2)
        emit_x_dmas(3, range(NCH))
        batch_section(3)

        # W_v after all x.  ctx half (rows 0:H) first - its matmuls wait on
        # batch 3's context, ready right after the last x tile - then the
        # always-ready h_t half chases the stream per k-block.
        wv0_sb = wts.tile([128, 8, SIZE], F16, tag="w0")
        for k in range(8):
            nc.sync.dma_start(
                out=wv0_sb[:, k : k + 1, :],
                in_=w_v[128 * k : 128 * (k + 1), :]
                .rearrange("(k p) o -> p k o", p=128),
            )
        wv1_sb = wts.tile([128, 8, SIZE], F16, tag="w1")
        for k in range(8):
            nc.sync.dma_start(
                out=wv1_sb[:, k : k + 1, :],
                in_=w_v[H + 128 * k : H + 128 * (k + 1), :]
                .rearrange("(k p) o -> p k o", p=128),
            )

        ps_o = psO.tile([BPC, SIZE], F32, tag="po", name="ps_o")
        for k in range(8):
            lhs = combT[:, 4 * k : 4 * k + 4]
            for h2 in range(2):
                nc.tensor.matmul(
                    ps_o[:, 512 * h2 : 512 * (h2 + 1)],
                    lhs,
                    wv0_sb[:, k, 512 * h2 : 512 * (h2 + 1)],
                    start=(k == 0),
                    stop=False,
                )
        for k in range(8, 16):
            lhs = combT[:, 4 * k : 4 * k + 4]
            for h2 in range(2):
                nc.tensor.matmul(
                    ps_o[:, 512 * h2 : 512 * (h2 + 1)],
                    lhs,
                    wv1_sb[:, k % 8, 512 * h2 : 512 * (h2 + 1)],
                    start=False,
                    stop=(k == 15),
                )
        # tanh+store in quarters so each store overlaps the next tanh
        Q = SIZE // 4
        for q in range(4):
            nc.scalar.activation(
                out=out_sb[:, Q * q : Q * (q + 1)],
                in_=ps_o[:, Q * q : Q * (q + 1)],
                func=AF.Tanh,
            )
            ring = nc.gpsimd if q % 2 == 0 else nc.sync
            ring.dma_start(
                out=outd[:, Q * q : Q * (q + 1)], in_=out_sb[:, Q * q : Q * (q + 1)]
            )

    nc.compile()
    return nc


def _host_prep(x, W_p, v_p, W_a, W_v):
    x = np.asarray(x, dtype=np.float32)
    h_all = np.ascontiguousarray(x[:, -1, :])  # [B, H] exact fp32 h_t
    x = np.ascontiguousarray(x.astype(np.float16))
    W_p = np.ascontiguousarray(np.asarray(W_p, dtype=np.float16))
    v_p = np.asarray(v_p, dtype=np.float32).reshape(-1)
    W_aT = np.ascontiguousarray(np.asarray(W_a, dtype=np.float32).T.astype(np.float16))
    W_v = np.ascontiguousarray(np.asarray(W_v, dtype=np.float16))
    vrep = np.ascontiguousarray(np.broadcast_to(v_p.reshape(1, H), (BPC, H)))
    cols = np.arange(COLS)
    p = np.arange(128)
    pos = ((cols[None, :] // A) * SCH + p[:, None] * A + (cols[None, :] % A)).astype(
        np.float32
    )
    pos = np.ascontiguousarray(
        np.concatenate([pos, np.ones((128, 1), np.float32)], axis=1)
    )
    ident = np.eye(128, dtype=np.float32)
    sel16 = np.zeros((BPC, BPC * 128), dtype=np.float16)
    for b in range(BPC):
        sel16[b, 128 * b : 128 * (b + 1)] = 1.0

    in_maps = []
    for c in range(NCORES):
        hT = h_all[BPC * c : BPC * (c + 1)].T.astype(np.float16)  # [H, BPC]
        htk_a = np.ascontiguousarray(
            hT.reshape(8, 128, BPC).transpose(1, 0, 2).reshape(128, 8 * BPC)
        )
        in_maps.append(
            dict(
                x_s=np.ascontiguousarray(x[BPC * c : BPC * (c + 1)]),
                w_p=W_p,
                w_at=W_aT,
                w_v=W_v,
                htk=htk_a,
                vrep=vrep,
                pos=pos,
                ident=ident,
                sel16=sel16,
            )
        )
    return in_maps


def kernel(x, W_p, v_p, W_a, W_v):
    if "nc" not in _CACHE:
        _CACHE["nc"] = _build()
    nc = _CACHE["nc"]
    in_maps = _host_prep(x, W_p, v_p, W_a, W_v)
    res = run_bass_kernel_spmd(nc, in_maps, core_ids=list(range(NCORES)), trace=TRACE)
    _CACHE["last_results"] = res
    return np.concatenate([r["out"] for r in res.results], axis=0)


# revision 19
# speedup vs baseline: 1.0446x; 1.0446x over previous
"""Trainium2 Bass kernel: Luong-style attention with predictive alignment.

Math (see reference):
    h_t    = x[:, -1, :]                                   [B, H]
    t      = tanh(h_t @ W_p);  aligned = S*sigmoid(t @ v_p)
    scores[b,s] = sum_h x[b,s,h] * u[b,h],  u[b] = W_a @ h_t[b]
        (algebraic rewrite of (x @ W_a) . h_t -- avoids the B*S*H*H einsum)
    attn   = softmax(scores) * exp(-(pos-aligned)^2 / sigma2)
    ctx[b] = sum_s attn[b,s] * x[b,s,:]
    out    = tanh(concat(ctx, h_t) @ W_v)

Sharding: data-parallel over batch. 8 cores x 4 batches each; weights
replicated per core.

The kernel is DMA-roofline-shaped: 24.8MB/core (x fp16 16.8 + weights 8)
at ~360GB/s is ~69us, so every engine's per-batch work must fit under the
~11.7us/batch x-stream pace.  Three structural choices make that true:

1. Score dot products (the dominant elementwise work, 16 cols x
   [128,1024] per batch) are split across TWO engines: 10 columns on DVE
   and 6 on GpSimd (both run scalar_tensor_tensor; the op only has a 1x
   DVE uop so a second engine beats any single-engine scheme).
2. Context is accumulated PER CHUNK in PSUM with UNNORMALIZED bfloat16
   attention weights at = e^{s-m1} * G (bf16 carries fp32's exponent
   range, so the e^{+38} worst case cannot overflow; PE accepts mixed
   bf16 lhsT x fp16 rhs).  The 1/Z normalization folds into the single
   [1,H] PSUM->SBUF context copy at the end, and Z itself is reduced
   across partitions by a tiny PE matmul against a ones column instead
   of a gpsimd all-reduce.  This deletes the old 32-matmul post-zinv
   context burst from the tail.
3. u[b] broadcast: u = W_a h_t is computed once for all 4 batches as a
   [4,H] PE matmul, then broadcast to 128 partitions by a rank-1 PE
   matmul against a ones row - ~5us of PE instead of ~16us.

DMA ring order (sync queue): W_aT k0-3 | x b0c0 | W_aT k4-7 | x b0c1-2 |
W_p | x b0c3-7 | x b1 | x b2 | x b3 | W_v[:H] per-k | W_v[H:] per-k.
The ctx half of W_v streams first so its matmuls (gated on batch 3's
context, ready just after the last x tile) can chase the stream; the
always-ready h_t half lands last.  W_v halves reuse W_aT's and W_p's
SBUF slots.  Small inputs ride the otherwise idle gpsimd ring.

Exact softmax: e^{s-g2-m1}/sum(e^{s-m1}) == softmax(s)*gauss for any
bias m1 (we use max over cols 0:8, fixed after chunk 3).  sigmoid(z) is
1/(1+e^{-z}) so the scalar engine never swaps activation tables
(Sigmoid lives in a different table set than Exp/Square/Tanh).
"""

import math
from contextlib import ExitStack

import numpy as np

import concourse.bass as bass
import concourse.bass_isa as bass_isa
import concourse.mybir as mybir
import concourse.tile as tile
from concourse import bacc
from concourse.bass_utils import run_bass_kernel_spmd

B, S, H, SIZE = 32, 2048, 1024, 1024
NCORES = 8
BPC = B // NCORES          # batches per core
NCH = 8                    # x chunks per batch
SCH = S // NCH             # 256 sequence positions per chunk
A = 2                      # sub-slices (128 s-positions each) per chunk
COLS = NCH * A             # 16 score columns per batch
F32 = mybir.dt.float32
F16 = mybir.dt.float16
BF16 = mybir.dt.bfloat16
SIGMA_SQ = 2.0 * (S / 2.0 / 2.0) ** 2    # D = S//2; 2*(D/2)^2 = 524288
INV_SG = 1.0 / math.sqrt(SIGMA_SQ)

# Score columns on the 1x-uop STT path (DVE alone, 1138ns/col).  The rest
# run as a 2x-mode DVE tensor_tensor multiply (594ns) + a Scalar-engine
# Copy-with-accumulator reduction (~800ns) - two engines sharing the
# dominant dot-product work.  Chunk 7 stays pure-DVE for tail latency.
STT_COLS = frozenset((0, 6, 8, 10, 14, 15))

_CACHE = {}
TRACE = False


def _build():
    AF = mybir.ActivationFunctionType
    OP = mybir.AluOpType
    nc = bacc.Bacc()

    x_s = nc.dram_tensor("x_s", [BPC, S, H], F16, kind="ExternalInput")
    w_p = nc.dram_tensor("w_p", [H, H], F16, kind="ExternalInput")
    w_at = nc.dram_tensor("w_at", [H, H], F16, kind="ExternalInput")
    w_v = nc.dram_tensor("w_v", [2 * H, SIZE], F16, kind="ExternalInput")
    htk = nc.dram_tensor("htk", [128, 8 * BPC], F16, kind="ExternalInput")
    vrep = nc.dram_tensor("vrep", [BPC, H], F32, kind="ExternalInput")
    posd = nc.dram_tensor("pos", [128, COLS + 1], F32, kind="ExternalInput")
    idd = nc.dram_tensor("ident", [128, 128], F32, kind="ExternalInput")
    seld = nc.dram_tensor("sel16", [BPC, BPC * 128], F16, kind="ExternalInput")
    outd = nc.dram_tensor("out", [BPC, SIZE], F32, kind="ExternalOutput")

    with tile.TileContext(nc) as tc, ExitStack() as ctx:
        const = ctx.enter_context(tc.tile_pool(name="const", bufs=1))
        wts = ctx.enter_context(tc.tile_pool(name="wts", bufs=1))
        xs = ctx.enter_context(tc.tile_pool(name="xs", bufs=16))
        ubcp = ctx.enter_context(tc.tile_pool(name="ubcp", bufs=4))
        ctxp = ctx.enter_context(tc.tile_pool(name="ctxp", bufs=2))
        prodp = ctx.enter_context(tc.tile_pool(name="prodp", bufs=6))
        prodg = ctx.enter_context(tc.tile_pool(name="prodg", bufs=2))
        small = ctx.enter_context(tc.tile_pool(name="small", bufs=2))
        psUbc = ctx.enter_context(
            tc.tile_pool(name="psUbc", bufs=1, space=bass.MemorySpace.PSUM)
        )
        psCtx = ctx.enter_context(
            tc.tile_pool(name="psCtx", bufs=1, space=bass.MemorySpace.PSUM)
        )
        psT = ctx.enter_context(
            tc.tile_pool(name="psT", bufs=1, space=bass.MemorySpace.PSUM)
        )
        psO = ctx.enter_context(
            tc.tile_pool(name="psO", bufs=1, space=bass.MemorySpace.PSUM)
        )
        dpool = ctx.enter_context(
            tc.tile_pool(name="dram", bufs=1, space=bass.MemorySpace.DRAM)
        )

        # ---- small inputs ride the gpsimd ring; bulk traffic owns sync ----
        combT = const.tile([128, 8 * BPC * 2], F16)  # combined^T: [p, 4k+b]
        v_sb = const.tile([BPC, H], F32)
        pos_sb = const.tile([128, COLS + 1], F32)    # last col = 1.0 (Z-sum)
        id_sb = const.tile([128, 128], F32)
        sel16 = const.tile([BPC, BPC * 128], F16)
        u16 = const.tile([BPC, H], F16)
        tta = const.tile([BPC, H], F32)
        alb = const.tile([BPC, 1], F32)
        out_sb = const.tile([BPC, SIZE], F32)

        nc.gpsimd.dma_start(out=combT[:, 32:64], in_=htk[:, :])
        nc.gpsimd.dma_start(out=v_sb, in_=vrep[:, :])
        nc.gpsimd.dma_start(out=pos_sb, in_=posd[:, :])
        nc.gpsimd.dma_start(out=id_sb, in_=idd[:, :])
        nc.gpsimd.dma_start(out=sel16, in_=seld[:, :])
        ones32 = pos_sb[:, COLS : COLS + 1]

        # ---- bulk ring: W_aT first half, then first x tile, etc ----
        all_x = [[None] * NCH for _ in range(BPC)]

        def emit_x_dmas(b, cs):
            for c in cs:
                xt = xs.tile([128, A, H], F16, tag="xt", name=f"xt_{b}_{c}")
                nc.sync.dma_start(
                    out=xt,
                    in_=x_s[b, c * SCH : (c + 1) * SCH, :]
                    .rearrange("(p a) h -> p a h", p=128),
                )
                all_x[b][c] = xt

        ubc_tiles = [None] * BPC

        # batch 0's u-broadcast goes the DIRECT route (lhsT = h_t column
        # replicated along its free dim by a 0-stride AP, so out[p,h] =
        # u[0,h] for every p) with the k-halves pipelined behind the two
        # W_aT DMA halves - no u16 intermediate, shortest possible chain
        # to the first score op.  Batches 1-3 use the cheap selector path.
        wa_sb = wts.tile([128, 8, H], F16, tag="w1")
        ub0_ps = psUbc.tile([128, H], F32, tag="ub", name="ubps_0")

        def emit_ubc0_half(ks):
            for k in ks:
                c0 = combT[:, 32 + 4 * k : 32 + 4 * k + 1]
                lhs = bass.AP(
                    tensor=c0.tensor, offset=c0.offset, ap=[c0.ap[0], [0, 128]]
                )
                for h2 in range(2):
                    nc.tensor.matmul(
                        ub0_ps[:, 512 * h2 : 512 * (h2 + 1)],
                        lhs,
                        wa_sb[:, k, 512 * h2 : 512 * (h2 + 1)],
                        start=(k == 0),
                        stop=(k == 7),
                    )

        nc.sync.dma_start(
            out=wa_sb[:, 0:4, :],
            in_=w_at[0 : H // 2, :].rearrange("(k p) j -> p k j", p=128),
        )
        emit_x_dmas(0, [0])
        emit_ubc0_half(range(4))
        nc.sync.dma_start(
            out=wa_sb[:, 4:8, :],
            in_=w_at[H // 2 :, :].rearrange("(k p) j -> p k j", p=128),
        )
        emit_ubc0_half(range(4, 8))
        ubc0 = ubcp.tile([128, H], F16, tag="ubc", name="ubc_0")
        nc.scalar.copy(ubc0[:, 0 : H // 2], ub0_ps[:, 0 : H // 2])
        nc.vector.tensor_copy(out=ubc0[:, H // 2 : H], in_=ub0_ps[:, H // 2 : H])
        ubc_tiles[0] = ubc0

        # u = W_a @ h_t for batches 1-3 in one [4, H] matmul set
        u_ps = psO.tile([BPC, H], F32, tag="po", name="u_ps")
        for k in range(8):
            lhs = combT[:, 32 + 4 * k : 32 + 4 * k + 4]
            for h2 in range(2):
                nc.tensor.matmul(
                    u_ps[:, 512 * h2 : 512 * (h2 + 1)],
                    lhs,
                    wa_sb[:, k, 512 * h2 : 512 * (h2 + 1)],
                    start=(k == 0),
                    stop=(k == 7),
                )
        nc.scalar.copy(u16[:, 0 : H // 2], u_ps[:, 0 : H // 2])
        nc.vector.tensor_copy(out=u16[:, H // 2 : H], in_=u_ps[:, H // 2 : H])

        emit_x_dmas(0, [1, 2])
        wp_sb = wts.tile([128, 8, H], F16, tag="w0")
        nc.sync.dma_start(
            out=wp_sb, in_=w_p[:, :].rearrange("(k p) j -> p k j", p=128)
        )
        emit_x_dmas(0, range(3, NCH))

        # broadcast u[b] to all 128 partitions: selector matmul (row b of
        # sel16[:, 128b:128(b+1)] is all-ones) against u16 at base partition 0
        for b in range(1, BPC):
            ub_ps = psUbc.tile([128, H], F32, tag="ub", name=f"ubps_{b}")
            for h2 in range(2):
                nc.tensor.matmul(
                    ub_ps[:, 512 * h2 : 512 * (h2 + 1)],
                    sel16[:, 128 * b : 128 * (b + 1)],
                    u16[:, 512 * h2 : 512 * (h2 + 1)],
                    start=True,
                    stop=True,
                )
            ubc = ubcp.tile([128, H], F16, tag="ubc", name=f"ubc_{b}")
            nc.scalar.copy(ubc[:, 0 : H // 2], ub_ps[:, 0 : H // 2])
            nc.vector.tensor_copy(out=ubc[:, H // 2 : H], in_=ub_ps[:, H // 2 : H])
            ubc_tiles[b] = ubc

        # ---- alignment: t = tanh(h_t@W_p); aligned = S*sigmoid(t@v_p) ----
        ab_d = dpool.tile([BPC, 1], F32)
        ab_tiles = [
            const.tile([128, 1], F32, name=f"abb_{bb}") for bb in range(BPC)
        ]

        def emit_aligned_section():
            ps_t = psO.tile([BPC, H], F32, tag="po", name="ps_t")
            for k in range(8):
                lhs = combT[:, 32 + 4 * k : 32 + 4 * k + 4]
                for h2 in range(2):
                    nc.tensor.matmul(
                        ps_t[:, 512 * h2 : 512 * (h2 + 1)],
                        lhs,
                        wp_sb[:, k, 512 * h2 : 512 * (h2 + 1)],
                        start=(k == 0),
                        stop=(k == 7),
                    )
            nc.scalar.activation(out=tta, in_=ps_t, func=AF.Tanh)

            prod0 = prodp.tile([BPC, H], F32, tag="pal")
            al_r = small.tile([BPC, 1], F32, tag="alr")
            nc.vector.scalar_tensor_tensor(
                out=prod0,
                in0=tta,
                scalar=1.0,
                in1=v_sb,
                op0=OP.mult,
                op1=OP.mult,
                accum_out=al_r,
            )
            # sigmoid via resident Exp table (avoids ACT_TABLE_LOAD swaps)
            e_neg = small.tile([BPC, 1], F32, tag="eneg")
            nc.scalar.activation(out=e_neg, in_=al_r, func=AF.Exp, bias=0.0, scale=-1.0)
            e_p1 = small.tile([BPC, 1], F32, tag="ep1")
            nc.vector.tensor_scalar_add(e_p1, e_neg, 1.0)
            sigv = small.tile([BPC, 1], F32, tag="sigv")
            nc.vector.reciprocal(sigv, e_p1)
            nc.scalar.mul(alb, sigv, -float(S) * INV_SG)  # alb = -aligned/sg
            # roundtrip + per-batch broadcasts on the idle gpsimd ring
            nc.gpsimd.dma_start(out=ab_d[:, :], in_=alb)
            for bb in range(BPC):
                nc.gpsimd.dma_start(
                    out=ab_tiles[bb], in_=ab_d[bb : bb + 1, :].to_broadcast((128, 1))
                )

        # ---- per-batch: scores + per-chunk unnormalized bf16 context ----
        NCH0 = 4          # chunks covered by the m1 bias phase
        C0 = NCH0 * A     # cols 0..7

        def batch_section(b, after_scores=None):
            ubc = ubc_tiles[b]
            sc_b = small.tile([128, COLS], F32, tag="scb", name=f"scb_{b}")
            ps_c = psCtx.tile([1, H], F32, tag="pc", name=f"pc_{b}")

            tt_prods = {}

            def emit_score_mul(col):
                c, a = col // A, col % A
                prod = prodp.tile([128, H], F16, tag="p0", name=f"pr_{b}_{col}")
                if col in STT_COLS:
                    # fused multiply+reduce, 1x DVE uop
                    nc.vector.scalar_tensor_tensor(
                        out=prod,
                        in0=all_x[b][c][:, a, :],
                        scalar=1.0,
                        in1=ubc,
                        op0=OP.mult,
                        op1=OP.mult,
                        accum_out=sc_b[:, col : col + 1],
                    )
                else:
                    # 2x-mode DVE multiply; Scalar engine reduces later
                    nc.vector.tensor_mul(prod, all_x[b][c][:, a, :], ubc)
                    tt_prods[col] = prod

            def emit_score_accum(col):
                if col not in tt_prods:
                    return
                dump = prodg.tile([128, H], F16, tag="pq", name=f"pq_{b}_{col}")
                nc.scalar.activation(
                    out=dump,
                    in_=tt_prods.pop(col),
                    func=AF.Copy,
                    accum_out=sc_b[:, col : col + 1],
                )

            def emit_score(col):
                emit_score_mul(col)
                emit_score_accum(col)

            def emit_ctx_mms(c, at_ap, first, last):
                # 4 matmuls per chunk: 2 sub-slice cols x 2 H-halves
                for a in range(A):
                    for h2 in range(2):
                        nc.tensor.matmul(
                            ps_c[:, 512 * h2 : 512 * (h2 + 1)],
                            at_ap[:, a : a + 1],
                            all_x[b][c][:, a, 512 * h2 : 512 * (h2 + 1)],
                            start=(first and a == 0),
                            stop=(last and a == 1 and h2 == 1),
                        )

            for col in range(C0):
                emit_score(col)

            # bias m1 = max over cols 0..7, broadcast to all partitions
            mx_p = small.tile([128, 1], F32, tag="mxp", name=f"mxp_{b}")
            nc.vector.reduce_max(
                out=mx_p, in_=sc_b[:, 0:C0], axis=mybir.AxisListType.X
            )
            mcast = small.tile([128, 1], F32, tag="mcast", name=f"mcast_{b}")
            nc.gpsimd.partition_all_reduce(
                mcast, mx_p, channels=128, reduce_op=bass_isa.ReduceOp.max
            )
            negm = small.tile([128, 1], F32, tag="negm", name=f"negm_{b}")
            nc.scalar.mul(negm, mcast, -1.0)

            # deep lookahead: chunks 4-6's DVE multiplies go ahead of every
            # exp-dependent DVE op so the in-order DVE queue never stalls
            # on the mcast/negm/exp round trip; their Scalar accumulations
            # are deferred per-chunk so exp0-3 isn't stuck behind them in
            # the (also in-order) Scalar queue
            for col in range(C0, C0 + 3 * A):
                emit_score_mul(col)
            if after_scores is not None:
                after_scores()

            g2 = small.tile([128, COLS], F32, tag="g2", name=f"g2_{b}")
            nc.scalar.activation(
                out=g2, in_=pos_sb[:, 0:COLS], func=AF.Square,
                bias=ab_tiles[b], scale=INV_SG,
            )
            gss = small.tile([128, COLS], F32, tag="gss", name=f"gss_{b}")
            nc.scalar.activation(out=gss, in_=g2, func=AF.Exp, bias=0.0, scale=-1.0)

            ew_all = small.tile([128, COLS], F32, tag="ew", name=f"ew_{b}")
            zp0 = small.tile([128, 1], F32, tag="zp0", name=f"zp0_{b}")
            nc.scalar.activation(
                out=ew_all[:, 0:C0],
                in_=sc_b[:, 0:C0],
                func=AF.Exp,
                bias=negm,
                scale=1.0,
                accum_out=zp0,
            )
            # unnormalized attention weights; context fires per chunk.
            # chunks 0-3: e^{s-m1} <= 1 by construction -> fp16 (11-bit
            # mantissa); chunks 4-7 can reach e^{+38} -> bf16 (fp32's
            # exponent range; PE accepts mixed 16-bit lhsT/rhs dtypes)
            at8 = small.tile([128, C0], F16, tag="at8", name=f"at8_{b}")
            nc.vector.tensor_mul(at8, ew_all[:, 0:C0], gss[:, 0:C0])
            for c in range(NCH0):
                emit_ctx_mms(c, at8[:, A * c : A * (c + 1)], first=(c == 0), last=False)

            # chunks 4..6: per-chunk exp -> at -> context (DVE muls already
            # in flight, Scalar accums land just-in-time); chunk 7's score
            # ops (pure-DVE STT) go right before their use
            zrun = zp0
            for ch in range(NCH0, NCH):
                col0 = ch * A
                if ch == NCH - 1:
                    for col in range(col0, col0 + A):
                        emit_score(col)
                else:
                    for col in range(col0, col0 + A):
                        emit_score_accum(col)
                zpc = small.tile([128, 1], F32, tag=f"zp{ch}", name=f"zp{ch}_{b}")
                nc.scalar.activation(
                    out=ew_all[:, col0 : col0 + A],
                    in_=sc_b[:, col0 : col0 + A],
                    func=AF.Exp,
                    bias=negm,
                    scale=1.0,
                    accum_out=zpc,
                )
                at2 = small.tile([128, A], BF16, tag=f"at{ch}", name=f"at{ch}_{b}")
                nc.vector.tensor_mul(
                    at2, ew_all[:, col0 : col0 + A], gss[:, col0 : col0 + A]
                )
                emit_ctx_mms(ch, at2, first=False, last=(ch == NCH - 1))
                zn = small.tile([128, 1], F32, tag=f"zr{ch}", name=f"zr{ch}_{b}")
                nc.scalar.add(zn, zpc, zrun)
                zrun = zn

            # Z = sum_p zrun[p] via a tiny PE matmul against the ones col
            psz = psT.tile([1, 1], F32, tag="pz", name=f"pz_{b}")
            nc.tensor.matmul(psz, zrun, ones32, start=True, stop=True)
            zinv = small.tile([1, 1], F32, tag="zinv", name=f"zinv_{b}")
            nc.vector.reciprocal(zinv, psz)

            # ctx out of PSUM with 1/Z folded in (split scalar/vector),
            # transpose 128-blocks, scatter into combT's per-batch columns
            ctx_t = ctxp.tile([1, H], F32, tag="ctx", name=f"ctx_{b}")
            nc.scalar.mul(ctx_t[0:1, 0 : H // 2], ps_c[0:1, 0 : H // 2], zinv)
            nc.vector.tensor_scalar(
                out=ctx_t[0:1, H // 2 : H],
                in0=ps_c[0:1, H // 2 : H],
                scalar1=zinv,
                scalar2=None,
                op0=OP.mult,
            )
            ps_ct = psT.tile([128, 8], F32, tag="pt", name=f"pct_{b}")
            for k in range(8):
                nc.tensor.transpose(
                    ps_ct[:, k : k + 1],
                    ctx_t[0:1, 128 * k : 128 * (k + 1)],
                    id_sb[0:1, 0:1],
                )
            cT = combT[:, b : b + 1]
            comb_cols = bass.AP(
                tensor=cT.tensor, offset=cT.offset, ap=[cT.ap[0], [4, 8]]
            )
            nc.scalar.copy(comb_cols, ps_ct)

        batch_section(0, after_scores=emit_aligned_section)
        emit_x_dmas(1, range(NCH))
        batch_section(1)
        emit_x_dmas(2, range(NCH))
        batch_section(2)
        emit_x_dmas(3, range(NCH))
        batch_section(3)

        # W_v after all x.  ctx half (rows 0:H) first - its matmuls wait on
        # batch 3's context, ready right after the last x tile - then the
        # always-ready h_t half chases the stream per k-block.
        wv0_sb = wts.tile([128, 8, SIZE], F16, tag="w0")
        for k in range(8):
            nc.sync.dma_start(
                out=wv0_sb[:, k : k + 1, :],
                in_=w_v[128 * k : 128 * (k + 1), :]
                .rearrange("(k p) o -> p k o", p=128),
            )
        wv1_sb = wts.tile([128, 8, SIZE], F16, tag="w1")
        for k in range(8):
            nc.sync.dma_start(
                out=wv1_sb[:, k : k + 1, :],
                in_=w_v[H + 128 * k : H + 128 * (k + 1), :]
                .rearrange("(k p) o -> p k o", p=128),
            )

        ps_o = psO.tile([BPC, SIZE], F32, tag="po", name="ps_o")
        for k in range(8):
            lhs = combT[:, 4 * k : 4 * k + 4]
            for h2 in range(2):
                nc.tensor.matmul(
                    ps_o[:, 512 * h2 : 512 * (h2 + 1)],
                    lhs,
                    wv0_sb[:, k, 512 * h2 : 512 * (h2 + 1)],
                    start=(k == 0),
                    stop=False,
                )
        for k in range(8, 16):
            lhs = combT[:, 4 * k : 4 * k + 4]
            for h2 in range(2):
                nc.tensor.matmul(
                    ps_o[:, 512 * h2 : 512 * (h2 + 1)],
                    lhs,
                    wv1_sb[:, k % 8, 512 * h2 : 512 * (h2 + 1)],
                    start=False,
                    stop=(k == 15),
                )
        # tanh+store in quarters so each store overlaps the next tanh
        Q = SIZE // 4
        for q in range(4):
            nc.scalar.activation(
                out=out_sb[:, Q * q : Q * (q + 1)],
                in_=ps_o[:, Q * q : Q * (q + 1)],
                func=AF.Tanh,
            )
            ring = nc.gpsimd if q % 2 == 0 else nc.sync
            ring.dma_start(
                out=outd[:, Q * q : Q * (q + 1)], in_=out_sb[:, Q * q : Q * (q + 1)]
            )

    nc.compile()
    return nc


def _host_prep(x, W_p, v_p, W_a, W_v):
    x = np.asarray(x, dtype=np.float32)
    h_all = np.ascontiguousarray(x[:, -1, :])  # [B, H] exact fp32 h_t
    x = np.ascontiguousarray(x.astype(np.float16))
    W_p = np.ascontiguousarray(np.asarray(W_p, dtype=np.float16))
    v_p = np.asarray(v_p, dtype=np.float32).reshape(-1)
    W_aT = np.ascontiguousarray(np.asarray(W_a, dtype=np.float32).T.astype(np.float16))
    W_v = np.ascontiguousarray(np.asarray(W_v, dtype=np.float16))
    vrep = np.ascontiguousarray(np.broadcast_to(v_p.reshape(1, H), (BPC, H)))
    cols = np.arange(COLS)
    p = np.arange(128)
    pos = ((cols[None, :] // A) * SCH + p[:, None] * A + (cols[None, :] % A)).astype(
        np.float32
    )
    pos = np.ascontiguousarray(
        np.concatenate([pos, np.ones((128, 1), np.float32)], axis=1)
    )
    ident = np.eye(128, dtype=np.float32)
    sel16 = np.zeros((BPC, BPC * 128), dtype=np.float16)
    for b in range(BPC):
        sel16[b, 128 * b : 128 * (b + 1)] = 1.0

    in_maps = []
    for c in range(NCORES):
        hT = h_all[BPC * c : BPC * (c + 1)].T.astype(np.float16)  # [H, BPC]
        htk_a = np.ascontiguousarray(
            hT.reshape(8, 128, BPC).transpose(1, 0, 2).reshape(128, 8 * BPC)
        )
        in_maps.append(
            dict(
                x_s=np.ascontiguousarray(x[BPC * c : BPC * (c + 1)]),
                w_p=W_p,
                w_at=W_aT,
                w_v=W_v,
                htk=htk_a,
                vrep=vrep,
                pos=pos,
                ident=ident,
                sel16=sel16,
            )
        )
    return in_maps


def kernel(x, W_p, v_p, W_a, W_v):
    if "nc" not in _CACHE:
        _CACHE["nc"] = _build()
    nc = _CACHE["nc"]
    in_maps = _host_prep(x, W_p, v_p, W_a, W_v)
    res = run_bass_kernel_spmd(nc, in_maps, core_ids=list(range(NCORES)), trace=TRACE)
    _CACHE["last_results"] = res
    return np.concatenate([r["out"] for r in res.results], axis=0)


# revision 32
# speedup vs baseline: 1.1645x; 1.1148x over previous
"""Trainium2 Bass kernel: Luong-style attention with predictive alignment.

Math (see reference):
    h_t    = x[:, -1, :]                                   [B, H]
    t      = tanh(h_t @ W_p);  aligned = S*sigmoid(t @ v_p)
    scores[b,s] = sum_h x[b,s,h] * u[b,h],  u[b] = W_a @ h_t[b]
        (algebraic rewrite of (x @ W_a) . h_t -- avoids the B*S*H*H einsum)
    attn   = softmax(scores) * exp(-(pos-aligned)^2 / sigma2)
    ctx[b] = sum_s attn[b,s] * x[b,s,:]
    out    = tanh(concat(ctx, h_t) @ W_v)

Sharding: data-parallel over batch. 8 cores x 4 batches each; weights
replicated per core.

The kernel is DMA-roofline-shaped: 24.8MB/core (x fp16 16.8 + weights 8)
at ~360GB/s is ~69us, so every engine's per-batch work must fit under the
~11.7us/batch x-stream pace.  Three structural choices make that true:

1. Score dot products (the dominant elementwise work, 16 cols x
   [128,1024] per batch) are split across TWO engines: 10 columns on DVE
   and 6 on GpSimd (both run scalar_tensor_tensor; the op only has a 1x
   DVE uop so a second engine beats any single-engine scheme).
2. Context is accumulated PER CHUNK in PSUM with UNNORMALIZED bfloat16
   attention weights at = e^{s-m1} * G (bf16 carries fp32's exponent
   range, so the e^{+38} worst case cannot overflow; PE accepts mixed
   bf16 lhsT x fp16 rhs).  The 1/Z normalization folds into the single
   [1,H] PSUM->SBUF context copy at the end, and Z itself is reduced
   across partitions by a tiny PE matmul against a ones column instead
   of a gpsimd all-reduce.  This deletes the old 32-matmul post-zinv
   context burst from the tail.
3. u[b] broadcast: u = W_a h_t is computed once for all 4 batches as a
   [4,H] PE matmul, then broadcast to 128 partitions by a rank-1 PE
   matmul against a ones row - ~5us of PE instead of ~16us.

DMA ring order (sync queue): W_aT k0-3 | x b0c0 | W_aT k4-7 | x b0c1-2 |
W_p | x b0c3-7 | x b1 | x b2 | x b3 | W_v[:H] per-k | W_v[H:] per-k.
The ctx half of W_v streams first so its matmuls (gated on batch 3's
context, ready just after the last x tile) can chase the stream; the
always-ready h_t half lands last.  W_v halves reuse W_aT's and W_p's
SBUF slots.  Small inputs ride the otherwise idle gpsimd ring.

Exact softmax: e^{s-g2-m1}/sum(e^{s-m1}) == softmax(s)*gauss for any
bias m1 (we use max over cols 0:8, fixed after chunk 3).  sigmoid(z) is
1/(1+e^{-z}) so the scalar engine never swaps activation tables
(Sigmoid lives in a different table set than Exp/Square/Tanh).
"""

import math
from contextlib import ExitStack

import numpy as np

import concourse.bass as bass
import concourse.bass_isa as bass_isa
import concourse.mybir as mybir
import concourse.tile as tile
from concourse import bacc
from concourse.bass_utils import run_bass_kernel_spmd

B, S, H, SIZE = 32, 2048, 1024, 1024
NCORES = 8
BPC = B // NCORES          # batches per core
NCH = 8                    # x chunks per batch
SCH = S // NCH             # 256 sequence positions per chunk
A = 2                      # sub-slices (128 s-positions each) per chunk
COLS = NCH * A             # 16 score columns per batch
F32 = mybir.dt.float32
F16 = mybir.dt.float16
BF16 = mybir.dt.bfloat16
SIGMA_SQ = 2.0 * (S / 2.0 / 2.0) ** 2    # D = S//2; 2*(D/2)^2 = 524288
INV_SG = 1.0 / math.sqrt(SIGMA_SQ)

# Score columns on the 1x-uop STT path (DVE alone, ~1138ns/col).  The rest
# run as a 2x-mode DVE tensor_tensor multiply (~600ns) + a Scalar-engine
# Copy-with-accumulator reduction (~1223ns incl. accumulator read) - two
# engines sharing the dominant dot-product work.  7/9 balances
# DVE ~= Scalar.  Chunk 7 stays pure-DVE for tail latency.
STT_COLS = frozenset((0, 4, 6, 8, 10, 14, 15))

_CACHE = {}
TRACE = False


def _build():
    AF = mybir.ActivationFunctionType
    OP = mybir.AluOpType
    nc = bacc.Bacc()

    x_s = nc.dram_tensor("x_s", [BPC, S, H], F16, kind="ExternalInput")
    w_p = nc.dram_tensor("w_p", [H, H], F16, kind="ExternalInput")
    w_at = nc.dram_tensor("w_at", [H, H], F16, kind="ExternalInput")
    w_v = nc.dram_tensor("w_v", [2 * H, SIZE], F16, kind="ExternalInput")
    htk = nc.dram_tensor("htk", [128, 8 * BPC], F16, kind="ExternalInput")
    vrep = nc.dram_tensor("vrep", [BPC, H], F32, kind="ExternalInput")
    posd = nc.dram_tensor("pos", [128, COLS + 1], F32, kind="ExternalInput")
    idd = nc.dram_tensor("ident", [128, 128], F32, kind="ExternalInput")
    seld = nc.dram_tensor("sel16", [BPC, BPC * 128], F16, kind="ExternalInput")
    seld32 = nc.dram_tensor("sel32", [BPC, BPC * 128], F32, kind="ExternalInput")
    outd = nc.dram_tensor("out", [BPC, SIZE], F32, kind="ExternalOutput")

    with tile.TileContext(nc) as tc, ExitStack() as ctx:
        const = ctx.enter_context(tc.tile_pool(name="const", bufs=1))
        wts = ctx.enter_context(tc.tile_pool(name="wts", bufs=1))
        xs = ctx.enter_context(tc.tile_pool(name="xs", bufs=16))
        ubcp = ctx.enter_context(tc.tile_pool(name="ubcp", bufs=4))
        ctxp = ctx.enter_context(tc.tile_pool(name="ctxp", bufs=2))
        prodp = ctx.enter_context(tc.tile_pool(name="prodp", bufs=6))
        prodg = ctx.enter_context(tc.tile_pool(name="prodg", bufs=2))
        small = ctx.enter_context(tc.tile_pool(name="small", bufs=2))
        psUbc = ctx.enter_context(
            tc.tile_pool(name="psUbc", bufs=1, space=bass.MemorySpace.PSUM)
        )
        psCtx = ctx.enter_context(
            tc.tile_pool(name="psCtx", bufs=1, space=bass.MemorySpace.PSUM)
        )
        psT = ctx.enter_context(
            tc.tile_pool(name="psT", bufs=1, space=bass.MemorySpace.PSUM)
        )
        psO = ctx.enter_context(
            tc.tile_pool(name="psO", bufs=1, space=bass.MemorySpace.PSUM)
        )
        dpool = ctx.enter_context(
            tc.tile_pool(name="dram", bufs=1, space=bass.MemorySpace.DRAM)
        )

        # ---- small inputs ride the gpsimd ring; bulk traffic owns sync ----
        combT = const.tile([128, 8 * BPC * 2], F16)  # combined^T: [p, 4k+b]
        v_sb = const.tile([BPC, H], F32)
        pos_sb = const.tile([128, COLS + 1], F32)    # last col = 1.0 (Z-sum)
        id_sb = const.tile([128, 128], F32)
        sel16 = const.tile([BPC, BPC * 128], F16)
        sel32 = const.tile([BPC, BPC * 128], F32)
        u16 = const.tile([BPC, H], F16)
        tta = const.tile([BPC, H], F32)
        alb = const.tile([BPC, 1], F32)
        out_sb = const.tile([BPC, SIZE], F32)

        # htk gates the u-broadcast chain -> first in the sync ring (8KB);
        # the rest ride the (late-starting) gpsimd ring
        nc.sync.dma_start(out=combT[:, 32:64], in_=htk[:, :])
        nc.gpsimd.dma_start(out=sel16, in_=seld[:, :])
        nc.gpsimd.dma_start(out=sel32, in_=seld32[:, :])
        nc.gpsimd.dma_start(out=v_sb, in_=vrep[:, :])
        nc.gpsimd.dma_start(out=pos_sb, in_=posd[:, :])
        nc.gpsimd.dma_start(out=id_sb, in_=idd[:, :])
        ones32 = pos_sb[:, COLS : COLS + 1]

        # ---- bulk ring: W_aT first half, then first x tile, etc ----
        all_x = [[None] * NCH for _ in range(BPC)]

        def emit_x_dmas(b, cs):
            for c in cs:
                xt = xs.tile([128, A, H], F16, tag="xt", name=f"xt_{b}_{c}")
                nc.sync.dma_start(
                    out=xt,
                    in_=x_s[b, c * SCH : (c + 1) * SCH, :]
                    .rearrange("(p a) h -> p a h", p=128),
                )
                all_x[b][c] = xt

        ubc_tiles = [None] * BPC

        # batch 0's u-broadcast goes the DIRECT route (lhsT = h_t column
        # replicated along its free dim by a 0-stride AP, so out[p,h] =
        # u[0,h] for every p) with the k-halves pipelined behind the two
        # W_aT DMA halves - no u16 intermediate, shortest possible chain
        # to the first score op.  Batches 1-3 use the cheap selector path.
        wa_sb = wts.tile([128, 8, H], F16, tag="w1")
        ub0_ps = psUbc.tile([128, H], F32, tag="ub", name="ubps_0")

        def emit_ubc0_half(ks):
            for k in ks:
                c0 = combT[:, 32 + 4 * k : 32 + 4 * k + 1]
                lhs = bass.AP(
                    tensor=c0.tensor, offset=c0.offset, ap=[c0.ap[0], [0, 128]]
                )
                for h2 in range(2):
                    nc.tensor.matmul(
                        ub0_ps[:, 512 * h2 : 512 * (h2 + 1)],
                        lhs,
                        wa_sb[:, k, 512 * h2 : 512 * (h2 + 1)],
                        start=(k == 0),
                        stop=(k == 7),
                    )

        nc.sync.dma_start(
            out=wa_sb[:, 0:4, :],
            in_=w_at[0 : H // 2, :].rearrange("(k p) j -> p k j", p=128),
        )
        emit_x_dmas(0, [0])
        emit_ubc0_half(range(4))
        nc.sync.dma_start(
            out=wa_sb[:, 4:8, :],
            in_=w_at[H // 2 :, :].rearrange("(k p) j -> p k j", p=128),
        )
        emit_ubc0_half(range(4, 8))
        ubc0 = ubcp.tile([128, H], F16, tag="ubc", name="ubc_0")
        nc.scalar.copy(ubc0[:, 0 : H // 2], ub0_ps[:, 0 : H // 2])
        nc.vector.tensor_copy(out=ubc0[:, H // 2 : H], in_=ub0_ps[:, H // 2 : H])
        ubc_tiles[0] = ubc0

        # u = W_a @ h_t for batches 1-3 in one [4, H] matmul set
        u_ps = psO.tile([BPC, H], F32, tag="po", name="u_ps")
        for k in range(8):
            lhs = combT[:, 32 + 4 * k : 32 + 4 * k + 4]
            for h2 in range(2):
                nc.tensor.matmul(
                    u_ps[:, 512 * h2 : 512 * (h2 + 1)],
                    lhs,
                    wa_sb[:, k, 512 * h2 : 512 * (h2 + 1)],
                    start=(k == 0),
                    stop=(k == 7),
                )
        nc.scalar.copy(u16[:, 0 : H // 2], u_ps[:, 0 : H // 2])
        nc.vector.tensor_copy(out=u16[:, H // 2 : H], in_=u_ps[:, H // 2 : H])

        emit_x_dmas(0, [1, 2])
        wp_sb = wts.tile([128, 8, H], F16, tag="w0")
        nc.sync.dma_start(
            out=wp_sb, in_=w_p[:, :].rearrange("(k p) j -> p k j", p=128)
        )
        emit_x_dmas(0, range(3, NCH))

        # broadcast u[b] to all 128 partitions: selector matmul (row b of
        # sel16[:, 128b:128(b+1)] is all-ones) against u16 at base partition 0
        for b in range(1, BPC):
            ub_ps = psUbc.tile([128, H], F32, tag="ub", name=f"ubps_{b}")
            for h2 in range(2):
                nc.tensor.matmul(
                    ub_ps[:, 512 * h2 : 512 * (h2 + 1)],
                    sel16[:, 128 * b : 128 * (b + 1)],
                    u16[:, 512 * h2 : 512 * (h2 + 1)],
                    start=True,
                    stop=True,
                )
            ubc = ubcp.tile([128, H], F16, tag="ubc", name=f"ubc_{b}")
            nc.scalar.copy(ubc[:, 0 : H // 2], ub_ps[:, 0 : H // 2])
            nc.vector.tensor_copy(out=ubc[:, H // 2 : H], in_=ub_ps[:, H // 2 : H])
            ubc_tiles[b] = ubc

        # ---- alignment: t = tanh(h_t@W_p); aligned = S*sigmoid(t@v_p) ----
        ab4 = const.tile([128, BPC], F32)
        ab_tiles = [ab4[:, bb : bb + 1] for bb in range(BPC)]

        def emit_aligned_section():
            ps_t = psO.tile([BPC, H], F32, tag="po", name="ps_t")
            for k in range(8):
                lhs = combT[:, 32 + 4 * k : 32 + 4 * k + 4]
                for h2 in range(2):
                    nc.tensor.matmul(
                        ps_t[:, 512 * h2 : 512 * (h2 + 1)],
                        lhs,
                        wp_sb[:, k, 512 * h2 : 512 * (h2 + 1)],
                        start=(k == 0),
                        stop=(k == 7),
                    )
            nc.scalar.activation(out=tta, in_=ps_t, func=AF.Tanh)

            prod0 = prodp.tile([BPC, H], F32, tag="pal")
            al_r = small.tile([BPC, 1], F32, tag="alr")
            nc.vector.scalar_tensor_tensor(
                out=prod0,
                in0=tta,
                scalar=1.0,
                in1=v_sb,
                op0=OP.mult,
                op1=OP.mult,
                accum_out=al_r,
            )
            # sigmoid via resident Exp table (avoids ACT_TABLE_LOAD swaps)
            e_neg = small.tile([BPC, 1], F32, tag="eneg")
            nc.scalar.activation(out=e_neg, in_=al_r, func=AF.Exp, bias=0.0, scale=-1.0)
            e_p1 = small.tile([BPC, 1], F32, tag="ep1")
            nc.vector.tensor_scalar_add(e_p1, e_neg, 1.0)
            sigv = small.tile([BPC, 1], F32, tag="sigv")
            nc.vector.reciprocal(sigv, e_p1)
            nc.scalar.mul(alb, sigv, -float(S) * INV_SG)  # alb = -aligned/sg
            # partition-broadcast alb via selector matmuls (no DRAM trip)
            ps_ab = psT.tile([128, BPC], F32, tag="pt", name="ps_ab")
            for bb in range(BPC):
                nc.tensor.matmul(
                    ps_ab[:, bb : bb + 1],
                    sel32[:, 128 * bb : 128 * (bb + 1)],
                    alb,
                    start=True,
                    stop=True,
                )
            nc.scalar.copy(ab4, ps_ab)

        # ---- per-batch: scores + per-chunk unnormalized bf16 context ----
        NCH0 = 4          # chunks covered by the m1 bias phase
        C0 = NCH0 * A     # cols 0..7

        def batch_section(b, after_scores=None):
            ubc = ubc_tiles[b]
            sc_b = small.tile([128, COLS], F32, tag="scb", name=f"scb_{b}")
            ps_c = psCtx.tile([1, H], F32, tag="pc", name=f"pc_{b}")

            tt_prods = {}

            def emit_score_mul(col):
                c, a = col // A, col % A
                prod = prodp.tile([128, H], F16, tag="p0", name=f"pr_{b}_{col}")
                if col in STT_COLS:
                    # fused multiply+reduce, 1x DVE uop
                    nc.vector.scalar_tensor_tensor(
                        out=prod,
                        in0=all_x[b][c][:, a, :],
                        scalar=1.0,
                        in1=ubc,
                        op0=OP.mult,
                        op1=OP.mult,
                        accum_out=sc_b[:, col : col + 1],
                    )
                else:
                    # 2x-mode DVE multiply; Scalar engine reduces later
                    nc.vector.tensor_mul(prod, all_x[b][c][:, a, :], ubc)
                    tt_prods[col] = prod

            def emit_score_accum(col):
                if col not in tt_prods:
                    return
                dump = prodg.tile([128, H], F16, tag="pq", name=f"pq_{b}_{col}")
                nc.scalar.activation(
                    out=dump,
                    in_=tt_prods.pop(col),
                    func=AF.Copy,
                    accum_out=sc_b[:, col : col + 1],
                )

            def emit_score(col):
                emit_score_mul(col)
                emit_score_accum(col)

            def emit_ctx_mms(c, at_ap, first, last):
                # 4 matmuls per chunk: 2 sub-slice cols x 2 H-halves
                for a in range(A):
                    for h2 in range(2):
                        nc.tensor.matmul(
                            ps_c[:, 512 * h2 : 512 * (h2 + 1)],
                            at_ap[:, a : a + 1],
                            all_x[b][c][:, a, 512 * h2 : 512 * (h2 + 1)],
                            start=(first and a == 0),
                            stop=(last and a == 1 and h2 == 1),
                        )

            for col in range(C0):
                emit_score(col)

            # bias m1 = max over cols 0..7, broadcast to all partitions
            mx_p = small.tile([128, 1], F32, tag="mxp", name=f"mxp_{b}")
            nc.vector.reduce_max(
                out=mx_p, in_=sc_b[:, 0:C0], axis=mybir.AxisListType.X
            )
            mcast = small.tile([128, 1], F32, tag="mcast", name=f"mcast_{b}")
            nc.gpsimd.partition_all_reduce(
                mcast, mx_p, channels=128, reduce_op=bass_isa.ReduceOp.max
            )
            negm = small.tile([128, 1], F32, tag="negm", name=f"negm_{b}")
            nc.scalar.mul(negm, mcast, -1.0)

            # the alignment chain (batch 0) goes here: its DVE/scalar ops
            # overlap the bias round trip instead of queueing behind all of
            # batch 0's remaining score muls
            if after_scores is not None:
                after_scores()
            # deep lookahead: chunks 4-6's DVE multiplies go ahead of every
            # exp-dependent DVE op so the in-order DVE queue never stalls
            # on the mcast/negm/exp round trip; their Scalar accumulations
            # are deferred per-chunk so exp0-3 isn't stuck behind them in
            # the (also in-order) Scalar queue
            for col in range(C0, C0 + 3 * A):
                emit_score_mul(col)

            g2 = small.tile([128, COLS], F32, tag="g2", name=f"g2_{b}")
            nc.scalar.activation(
                out=g2, in_=pos_sb[:, 0:COLS], func=AF.Square,
                bias=ab_tiles[b], scale=INV_SG,
            )
            gss = small.tile([128, COLS], F32, tag="gss", name=f"gss_{b}")
            nc.scalar.activation(out=gss, in_=g2, func=AF.Exp, bias=0.0, scale=-1.0)

            ew_all = small.tile([128, COLS], F32, tag="ew", name=f"ew_{b}")
            zp0 = small.tile([128, 1], F32, tag="zp0", name=f"zp0_{b}")
            nc.scalar.activation(
                out=ew_all[:, 0:C0],
                in_=sc_b[:, 0:C0],
                func=AF.Exp,
                bias=negm,
                scale=1.0,
                accum_out=zp0,
            )
            # unnormalized attention weights; context fires per chunk.
            # chunks 0-3: e^{s-m1} <= 1 by construction -> fp16 (11-bit
            # mantissa); chunks 4-7 can reach e^{+38} -> bf16 (fp32's
            # exponent range; PE accepts mixed 16-bit lhsT/rhs dtypes)
            at8 = small.tile([128, C0], F16, tag="at8", name=f"at8_{b}")
            nc.vector.tensor_mul(at8, ew_all[:, 0:C0], gss[:, 0:C0])
            for c in range(NCH0):
                emit_ctx_mms(c, at8[:, A * c : A * (c + 1)], first=(c == 0), last=False)

            # chunks 4..6: per-chunk exp -> at -> context (DVE muls already
            # in flight, Scalar accums land just-in-time); chunk 7's score
            # ops (pure-DVE STT) go right before their use
            zrun = zp0
            for ch in range(NCH0, NCH):
                col0 = ch * A
                if ch == NCH - 1:
                    for col in range(col0, col0 + A):
                        emit_score(col)
                else:
                    for col in range(col0, col0 + A):
                        emit_score_accum(col)
                zpc = small.tile([128, 1], F32, tag=f"zp{ch}", name=f"zp{ch}_{b}")
                nc.scalar.activation(
                    out=ew_all[:, col0 : col0 + A],
                    in_=sc_b[:, col0 : col0 + A],
                    func=AF.Exp,
                    bias=negm,
                    scale=1.0,
                    accum_out=zpc,
                )
                at2 = small.tile([128, A], BF16, tag=f"at{ch}", name=f"at{ch}_{b}")
                nc.vector.tensor_mul(
                    at2, ew_all[:, col0 : col0 + A], gss[:, col0 : col0 + A]
                )
                emit_ctx_mms(ch, at2, first=False, last=(ch == NCH - 1))
                zn = small.tile([128, 1], F32, tag=f"zr{ch}", name=f"zr{ch}_{b}")
                nc.vector.tensor_add(zn, zpc, zrun)
                zrun = zn

            # Z = sum_p zrun[p] via a tiny PE matmul against the ones col
            psz = psT.tile([1, 1], F32, tag="pz", name=f"pz_{b}")
            nc.tensor.matmul(psz, zrun, ones32, start=True, stop=True)
            zinv = small.tile([1, 1], F32, tag="zinv", name=f"zinv_{b}")
            nc.vector.reciprocal(zinv, psz)

            # ctx out of PSUM with 1/Z folded in (split scalar/vector),
            # transpose 128-blocks, scatter into combT's per-batch columns
            ctx_t = ctxp.tile([1, H], F32, tag="ctx", name=f"ctx_{b}")
            nc.scalar.mul(ctx_t[0:1, 0 : H // 2], ps_c[0:1, 0 : H // 2], zinv)
            nc.vector.tensor_scalar(
                out=ctx_t[0:1, H // 2 : H],
                in0=ps_c[0:1, H // 2 : H],
                scalar1=zinv,
                scalar2=None,
                op0=OP.mult,
            )
            ps_ct = psT.tile([128, 8], F32, tag="pt", name=f"pct_{b}")
            for k in range(8):
                nc.tensor.transpose(
                    ps_ct[:, k : k + 1],
                    ctx_t[0:1, 128 * k : 128 * (k + 1)],
                    id_sb[0:1, 0:1],
                )
            cT = combT[:, b : b + 1]
            comb_cols = bass.AP(
                tensor=cT.tensor, offset=cT.offset, ap=[cT.ap[0], [4, 8]]
            )
            nc.scalar.copy(comb_cols, ps_ct)

        batch_section(0, after_scores=emit_aligned_section)
        emit_x_dmas(1, range(NCH))
        batch_section(1)
        emit_x_dmas(2, range(NCH))
        batch_section(2)
        emit_x_dmas(3, range(NCH))
        batch_section(3)

        # W_v after all x.  ctx half (rows 0:H) first - its matmuls wait on
        # batch 3's context, ready right after the last x tile - then the
        # always-ready h_t half chases the stream per k-block.
        wv0_sb = wts.tile([128, 8, SIZE], F16, tag="w0")
        for k in range(8):
            nc.sync.dma_start(
                out=wv0_sb[:, k : k + 1, :],
                in_=w_v[128 * k : 128 * (k + 1), :]
                .rearrange("(k p) o -> p k o", p=128),
            )
        wv1_sb = wts.tile([128, 8, SIZE], F16, tag="w1")
        for k in range(8):
            nc.sync.dma_start(
                out=wv1_sb[:, k : k + 1, :],
                in_=w_v[H + 128 * k : H + 128 * (k + 1), :]
                .rearrange("(k p) o -> p k o", p=128),
            )

        ps_o = psO.tile([BPC, SIZE], F32, tag="po", name="ps_o")
        for k in range(8):
            lhs = combT[:, 4 * k : 4 * k + 4]
            for h2 in range(2):
                nc.tensor.matmul(
                    ps_o[:, 512 * h2 : 512 * (h2 + 1)],
                    lhs,
                    wv0_sb[:, k, 512 * h2 : 512 * (h2 + 1)],
                    start=(k == 0),
                    stop=False,
                )
        for k in range(8, 16):
            lhs = combT[:, 4 * k : 4 * k + 4]
            for h2 in range(2):
                nc.tensor.matmul(
                    ps_o[:, 512 * h2 : 512 * (h2 + 1)],
                    lhs,
                    wv1_sb[:, k % 8, 512 * h2 : 512 * (h2 + 1)],
                    start=False,
                    stop=(k == 15),
                )
        # tanh+store in quarters so each store overlaps the next tanh
        Q = SIZE // 4
        for q in range(4):
            nc.scalar.activation(
                out=out_sb[:, Q * q : Q * (q + 1)],
                in_=ps_o[:, Q * q : Q * (q + 1)],
                func=AF.Tanh,
            )
            ring = nc.gpsimd if q % 2 == 0 else nc.sync
            ring.dma_start(
                out=outd[:, Q * q : Q * (q + 1)], in_=out_sb[:, Q * q : Q * (q + 1)]
            )

    nc.compile()
    return nc


def _host_prep(x, W_p, v_p, W_a, W_v):
    x = np.asarray(x, dtype=np.float32)
    h_all = np.ascontiguousarray(x[:, -1, :])  # [B, H] exact fp32 h_t
    x = np.ascontiguousarray(x.astype(np.float16))
    W_p = np.ascontiguousarray(np.asarray(W_p, dtype=np.float16))
    v_p = np.asarray(v_p, dtype=np.float32).reshape(-1)
    W_aT = np.ascontiguousarray(np.asarray(W_a, dtype=np.float32).T.astype(np.float16))
    W_v = np.ascontiguousarray(np.asarray(W_v, dtype=np.float16))
    vrep = np.ascontiguousarray(np.broadcast_to(v_p.reshape(1, H), (BPC, H)))
    cols = np.arange(COLS)
    p = np.arange(128)
    pos = ((cols[None, :] // A) * SCH + p[:, None] * A + (cols[None, :] % A)).astype(
        np.float32
    )
    pos = np.ascontiguousarray(
        np.concatenate([pos, np.ones((128, 1), np.float32)], axis=1)
    )
    ident = np.eye(128, dtype=np.float32)
    sel16 = np.zeros((BPC, BPC * 128), dtype=np.float16)
    for b in range(BPC):
        sel16[b, 128 * b : 128 * (b + 1)] = 1.0
    sel32 = sel16.astype(np.float32)

    in_maps = []
    for c in range(NCORES):
        hT = h_all[BPC * c : BPC * (c + 1)].T.astype(np.float16)  # [H, BPC]
        htk_a = np.ascontiguousarray(
            hT.reshape(8, 128, BPC).transpose(1, 0, 2).reshape(128, 8 * BPC)
        )
        in_maps.append(
            dict(
                x_s=np.ascontiguousarray(x[BPC * c : BPC * (c + 1)]),
                w_p=W_p,
                w_at=W_aT,
                w_v=W_v,
                htk=htk_a,
                vrep=vrep,
                pos=pos,
                ident=ident,
                sel16=sel16,
                sel32=sel32,
            )
        )
    return in_maps


def kernel(x, W_p, v_p, W_a, W_v):
    if "nc" not in _CACHE:
        _CACHE["nc"] = _build()
    nc = _CACHE["nc"]
    in_maps = _host_prep(x, W_p, v_p, W_a, W_v)
    res = run_bass_kernel_spmd(nc, in_maps, core_ids=list(range(NCORES)), trace=TRACE)
    _CACHE["last_results"] = res
    return np.concatenate([r["out"] for r in res.results], axis=0)
